# revision 6
# baseline (speedup 1.0000x reference)
"""KNN classification kernel for Trainium2 (8 NeuronCores), v2.

Problem: B=1024 queries x N=200000 gallery, D=256, top-10 neighbors,
softmax-weighted one-hot class scores over 50 classes.

Math fold: reference computes gallery = l2norm(train.T, axis=1) -- each
feature dim d is scaled by 1/||train[:, d]|| over the FULL gallery. That
folds into the query side, so the device only needs q_scaled @ train.T.

Device (per core, gallery sharded along N into 8 x 25000, zero-padded to
25088 = 24.5 granules x 1024):
  PE:  fp8e4 DoubleRow matmuls (K=256 packed as [128,2]) -> sim granule
       [128q, 1024] f32 in PSUM (0.5 cycles/row = 4x bf16 throughput)
  Screen (the bottleneck) split across two engines, alternating granules:
   - DVE tensor_reduce(max, axis=X) over [128,4,256] -> four per-256 maxes
   - ACT Relu(x - tau_b) with accum_out -> per-1024 exceedance sum, where
     tau_b = 3.25 * sigma_dev(b) is a per-query threshold (bias AP)
  PSUM ring: 4 granule buffers x 2 banks. Loop is gallery-block-major so
  each gallery DMA block is consumed by all 8 query chunks before the next
  block is needed (hides all gallery DMA after the first block).
Host: flag 256-blocks (DVE max >= tau) and 1024-granules (relu sum > 0),
  rescore flagged columns exactly in f64, exact top-10 -> softmax scores.
  Certificate: found 10th value must clear tau + 5.7 sigma_noise, else that
  query falls back to a full exact rescore (probability ~1e-7).
Safety: any exact-top-10 item has z >= ~3.8 sigma whp while tau = 3.25
  sigma; fp8 dot-product noise is ~0.06 sigma, so a top-10 item landing
  under the device-side threshold needs a ~10-sigma-noise deviation.
"""

import os
import numpy as np

NB_KNN = 10
T = 0.07
NUM_CLASSES = 50
EPS = 1e-12

B, N, D = 1024, 200000, 256
NCORES = 8
NPC = N // NCORES           # 25000 real cols per core
GR = 1024                   # granule width (2 PSUM banks)
NGR = 25                    # granules per chunk (last one is half width)
NPC_PAD = 25088             # 24 x 1024 + 512
NCH = 8                     # query chunks of 128
# gallery DMA blocks, in granules (last granule is 512 cols)
BLK_GR = [1, 4, 5, 5, 5, 4, 1]
SUB = 256                   # DVE max sub-block width
TAU_Z = 3.25                # screen threshold in device-sigma units
CERT_Z = 0.33               # certificate margin in device-sigma units

_CACHE = {}


def _gr_width(j):
    return 512 if j == NGR - 1 else GR


def _gr_col(j):
    return j * GR


def _build_bass(double_row=True):
    import concourse.bacc as bacc
    import concourse.tile as tile
    from concourse import mybir

    nc = bacc.Bacc("TRN2")
    f32 = mybir.dt.float32
    fp8 = mybir.dt.float8e4

    g_d = nc.dram_tensor("g", [128, 2, NPC_PAD], fp8, kind="ExternalInput")
    q_d = nc.dram_tensor("q", [128, 2, B], fp8, kind="ExternalInput")
    tau_d = nc.dram_tensor("tau", [128, NCH], f32, kind="ExternalInput")
    # per chunk: 13*4 per-256 maxes (DVE), 13 relu sums (ACT); half granule
    # 24 contributes 2 maxes or 1 sum depending on parity
    dve_d = nc.dram_tensor("dve", [128, NCH * 52], f32, kind="ExternalOutput")
    act_d = nc.dram_tensor("act", [128, NCH * 13], f32, kind="ExternalOutput")

    X = mybir.AxisListType.X
    MAX = mybir.AluOpType.max
    RELU = mybir.ActivationFunctionType.Relu
    pm = mybir.MatmulPerfMode.DoubleRow if double_row else None

    with tile.TileContext(nc) as tc:
        with tc.tile_pool(name="qp", bufs=1) as qp, \
             tc.tile_pool(name="gp", bufs=1) as gp, \
             tc.tile_pool(name="op", bufs=1) as op, \
             tc.tile_pool(name="pp", bufs=1, space="PSUM") as pp:
            q8 = qp.tile([128, 2, B], fp8, tag="q8")
            tau = qp.tile([128, NCH], f32, tag="tau")
            gal = []
            g0 = 0
            gtiles = []
            for i, ng in enumerate(BLK_GR):
                c0 = _gr_col(g0)
                cw = sum(_gr_width(g0 + k) for k in range(ng))
                t = gp.tile([128, 2, cw], fp8, tag=f"gal{i}", name=f"gal_t{i}")
                gal.append((t, g0, c0))
                gtiles.append((t, c0, cw))
                g0 += ng
            # DMA order tuned for pipeline head: first gallery block, the
            # first query chunk, then the rest
            t0, c00, cw0 = gtiles[0]
            nc.sync.dma_start(out=t0[:], in_=g_d[:, :, c00:c00 + cw0])
            nc.sync.dma_start(out=q8[:, :, 0:128], in_=q_d[:, :, 0:128])
            nc.sync.dma_start(out=q8[:, :, 128:B], in_=q_d[:, :, 128:B])
            nc.sync.dma_start(out=tau[:], in_=tau_d[:])
            for t, c0, cw in gtiles[1:]:
                nc.sync.dma_start(out=t[:], in_=g_d[:, :, c0:c0 + cw])

            dve_o = op.tile([128, NCH * 52], f32, tag="dve_o")
            act_o = op.tile([128, NCH * 13], f32, tag="act_o")

            for i, ng in enumerate(BLK_GR):
                t, gbase, cbase = gal[i]
                for c in range(NCH):
                    lhs = q8[:, :, c * 128:(c + 1) * 128]
                    for k in range(ng):
                        j = gbase + k
                        w = _gr_width(j)
                        l0 = _gr_col(j) - cbase
                        ps = pp.tile([128, GR], f32, tag="ps", bufs=4)
                        if double_row:
                            nc.tensor.matmul(ps[:, :w], lhs,
                                             t[:, :, l0:l0 + w],
                                             start=True, stop=True,
                                             perf_mode=pm)
                        else:
                            nc.tensor.matmul(ps[:, :w], lhs[:, 0],
                                             t[:, 0, l0:l0 + w],
                                             start=True, stop=False)
                            nc.tensor.matmul(ps[:, :w], lhs[:, 1],
                                             t[:, 1, l0:l0 + w],
                                             start=False, stop=True)
                        o = j // 2
                        nsub = w // SUB
                        if (j + c) % 2 == 0:
                            d0 = c * 52 + 4 * o
                            nc.vector.tensor_reduce(
                                dve_o[:, d0:d0 + nsub],
                                ps[:, :w].rearrange("p (r w) -> p r w",
                                                    r=nsub),
                                axis=X, op=MAX)
                        else:
                            a0 = c * 13 + o
                            nc.scalar.activation(
                                out=ps[:, :w], in_=ps[:, :w], func=RELU,
                                bias=tau[:, c:c + 1], scale=1.0,
                                accum_out=act_o[:, a0:a0 + 1])
            h52 = (NCH // 2) * 52
            h13 = (NCH // 2) * 13
            nc.sync.dma_start(out=dve_d[:, :h52], in_=dve_o[:, :h52])
            nc.sync.dma_start(out=act_d[:, :h13], in_=act_o[:, :h13])
            nc.sync.dma_start(out=dve_d[:, h52:], in_=dve_o[:, h52:])
            nc.sync.dma_start(out=act_d[:, h13:], in_=act_o[:, h13:])
    if not nc.is_finalized():
        nc.finalize()
    return nc


def _run_device(g_shards, q_packed, tau_packed):
    from concourse.bass_utils import run_bass_kernel_spmd
    if "nc" not in _CACHE:
        _CACHE["nc"] = _build_bass()
    nc = _CACHE["nc"]
    in_maps = [{"g": g_shards[c], "q": q_packed, "tau": tau_packed}
               for c in range(NCORES)]
    res = run_bass_kernel_spmd(nc, in_maps, list(range(NCORES)))
    return ([res.results[c]["dve"] for c in range(NCORES)],
            [res.results[c]["act"] for c in range(NCORES)])


def _run_emulated(g_shards, q_packed, tau_packed):
    """Numpy emulation of the device kernel (same outputs)."""
    qf = q_packed.astype(np.float32)         # [128, 2, B]
    dves, acts = [], []
    for core in range(NCORES):
        gf = g_shards[core].astype(np.float32)   # [128, 2, NPC_PAD]
        sim = np.einsum("pib,pin->bn", qf, gf)   # [B, NPC_PAD]
        dve = np.zeros((NCH, 128, 52), np.float32)
        act = np.zeros((NCH, 128, 13), np.float32)
        for c in range(NCH):
            sc = sim[c * 128:(c + 1) * 128]      # [128, NPC_PAD]
            for j in range(NGR):
                o = j // 2
                w = _gr_width(j)
                gsl = sc[:, _gr_col(j):_gr_col(j) + w]
                if (j + c) % 2 == 0:
                    nsub = w // SUB
                    m = gsl.reshape(128, nsub, SUB).max(axis=2)
                    dve[c, :, 4 * o:4 * o + nsub] = m
                else:
                    bias = tau_packed[:, c:c + 1]
                    act[c, :, o] = np.maximum(gsl + bias, 0).sum(axis=1)
        dves.append(dve)
        acts.append(act)
    return dves, acts


def kernel(test_features, train_features, train_labels):
    import ml_dtypes
    FP8 = ml_dtypes.float8_e4m3fn

    test_features = np.asarray(test_features, dtype=np.float32)
    train_features = np.asarray(train_features, dtype=np.float32)
    labels = np.asarray(train_labels).astype(np.int64)

    # ---- host pre: fold normalizations into the query side ----
    tf64 = train_features.astype(np.float64)             # [N, D]
    norm_d = np.maximum(np.sqrt(np.sum(tf64 * tf64, axis=0)), EPS)
    q64 = test_features.astype(np.float64)
    qn = np.sqrt(np.sum(q64 * q64, axis=1, keepdims=True))
    q_scaled = q64 / np.maximum(qn, EPS) / norm_d        # [B, D] f64

    # per-query fp8 scale so entries have rms ~8 (well inside e4m3 range)
    sigma_b = np.sqrt(np.sum(q_scaled * q_scaled, axis=1))   # exact sim std
    s_b = 128.0 / sigma_b                                 # [B]
    q8 = (q_scaled * s_b[:, None]).astype(FP8)            # [B, D]
    g8 = train_features.T.astype(FP8)                     # [D, N]

    # device-side sim std (from the actual quantized values)
    q8f = q8.astype(np.float64)
    g8_sq_mean = float(np.mean(g8.astype(np.float32) ** 2))
    sig_dev = np.sqrt(np.sum(q8f * q8f, axis=1) * g8_sq_mean)  # [B]
    tau_dev = TAU_Z * sig_dev                             # [B]

    # ---- pack device inputs ----
    q_packed = np.ascontiguousarray(
        q8.T.reshape(2, 128, B).transpose(1, 0, 2))       # [128, 2, B]
    g_shards = []
    for core in range(NCORES):
        sl = np.zeros((2, 128, NPC_PAD), dtype=FP8)
        sl[:, :, :NPC] = g8[:, core * NPC:(core + 1) * NPC].reshape(2, 128, NPC)
        g_shards.append(np.ascontiguousarray(sl.transpose(1, 0, 2)))
    tau_packed = np.ascontiguousarray(
        (-tau_dev).astype(np.float32).reshape(NCH, 128).T)    # [128, NCH]

    # ---- device: fp8 matmul + 2-engine screen ----
    if os.environ.get("KNN_EMULATE"):
        dves, acts = _run_emulated(g_shards, q_packed, tau_packed)
    else:
        dves, acts = _run_device(g_shards, q_packed, tau_packed)

    # ---- host: flag 256-blocks, exact f64 rescore, top-10, softmax ----
    NBLK = NPC_PAD // SUB                                 # 98 per core
    flags = np.zeros((B, NCORES, NBLK), dtype=bool)
    for core in range(NCORES):
        dve = dves[core].astype(np.float64)
        act = acts[core].astype(np.float64)
        if dve.ndim == 2:    # [128, NCH*52] device layout -> [NCH,128,52]
            dve = dve.reshape(128, NCH, 52).transpose(1, 0, 2)
            act = act.reshape(128, NCH, 13).transpose(1, 0, 2)
        for c in range(NCH):
            brow = slice(c * 128, (c + 1) * 128)
            tt = tau_dev[brow]                            # [128]
            for j in range(NGR):
                o = j // 2
                w = _gr_width(j)
                nsub = w // SUB
                k0 = _gr_col(j) // SUB
                if (j + c) % 2 == 0:
                    m = dve[c, :, 4 * o:4 * o + nsub]     # [128, nsub]
                    flags[brow, core, k0:k0 + nsub] |= m >= tt[:, None]
                else:
                    f = act[c, :, o] > 0.0
                    flags[brow, core, k0:k0 + nsub] |= f[:, None]

    flags = flags.reshape(B, NCORES * NBLK)
    seg_queries = [np.nonzero(flags[:, s])[0] for s in range(NCORES * NBLK)]

    per_q_vals = [[] for _ in range(B)]
    per_q_cols = [[] for _ in range(B)]
    for s, qs in enumerate(seg_queries):
        if len(qs) == 0:
            continue
        core, k = divmod(s, NBLK)
        c0 = core * NPC + SUB * k
        c1 = core * NPC + min(SUB * k + SUB, NPC)
        if c0 >= c1:
            continue
        block = tf64[c0:c1]                               # [w, D]
        sims = q_scaled[qs] @ block.T                     # [nq, w] f64
        cols = np.arange(c0, c1)
        for i, b in enumerate(qs):
            per_q_vals[b].append(sims[i])
            per_q_cols[b].append(cols)

    scores = np.zeros((B, NUM_CLASSES), dtype=np.float64)
    fallback = []
    for b in range(B):
        if per_q_vals[b]:
            v = np.concatenate(per_q_vals[b])
            cidx = np.concatenate(per_q_cols[b])
        else:
            v = np.empty(0)
            cidx = np.empty(0, np.int64)
        if len(v) < NB_KNN:
            fallback.append(b)
            continue
        sel = np.argpartition(-v, NB_KNN - 1)[:NB_KNN]
        # certificate: 10th best must clear tau + noise margin (device units)
        v10_dev = s_b[b] * np.sort(v[sel])[0]
        if v10_dev <= tau_dev[b] + CERT_Z * sig_dev[b]:
            fallback.append(b)
            continue
        order = np.lexsort((cidx[sel], -v[sel]))
        sel = sel[order]
        topv = v[sel]
        w = np.exp(topv / T - np.max(topv) / T)
        w /= w.sum()
        np.add.at(scores[b], labels[cidx[sel]], w)

    if fallback:
        fb = np.asarray(fallback)
        sims = q_scaled[fb] @ tf64.T                      # [nfb, N] f64
        for i, b in enumerate(fb):
            v = sims[i]
            sel = np.argpartition(-v, NB_KNN - 1)[:NB_KNN]
            order = np.lexsort((sel, -v[sel]))
            sel = sel[order]
            topv = v[sel]
            w = np.exp(topv / T - np.max(topv) / T)
            w /= w.sum()
            np.add.at(scores[b], labels[sel], w)

    return scores.astype(np.float32)


if __name__ == "__main__":
    rng = np.random.default_rng(0)
    tf = rng.standard_normal((B, D), dtype=np.float32)
    trf = rng.standard_normal((N, D), dtype=np.float32)
    trl = rng.integers(0, NUM_CLASSES, N).astype(np.int64)
    os.environ["KNN_EMULATE"] = "1"
    out = kernel(tf, trf, trl)
    print(out.shape, out.dtype, out.sum())


# revision 8
# speedup vs baseline: 1.0516x; 1.0516x over previous
"""KNN kernel for Trainium2, v3 ("Option C").

Same structure as v2 (fp8 DoubleRow matmul + 2-engine screen), but:
 - ACT writes relu(x - tau_b) as fp8 to SBUF (no accum_out, saving the
   187ns read-accumulator cost per instruction); bytes are DMA'd out and
   the host finds candidate columns directly from nonzero bytes.
 - granule -> engine assignment is a greedy global load balance over the
   exact per-instruction costs, so DVE and ACT stay in lockstep within
   every (gallery block x chunk) phase.
"""

import os
import numpy as np

NB_KNN = 10
T = 0.07
NUM_CLASSES = 50
EPS = 1e-12

B, N, D = 1024, 200000, 256
NCORES = 8
NPC = N // NCORES           # 25000 real cols per core
GR = 1024                   # granule width (2 PSUM banks)
NGR = 25                    # granules per chunk (last one is half width)
NPC_PAD = 25088             # 24 x 1024 + 512
NCH = 8                     # query chunks of 128
# gallery DMA blocks as (first granule, count); the half granule (24)
# leads so the run starts fast and ends with a big block that hides the
# output-DMA drain
BLOCKS = [(24, 1), (0, 4), (4, 5), (9, 5), (14, 5), (19, 5)]
SUB = 256                   # DVE max sub-block width
TAU_Z = 3.25                # screen threshold in device-sigma units
CERT_Z = 0.33               # certificate margin in device-sigma units

_CACHE = {}


def _gr_width(j):
    return 512 if j == NGR - 1 else GR


def _build_schedule():
    """Greedy global load balance of granules onto DVE ('D') and ACT ('A').
    Walks granules in device execution order keeping both engines' total
    assigned time equal, so every phase stays balanced."""
    D_FULL, D_HALF = 1192, 658
    A_FULL, A_HALF = 1085, 612
    tD = tA = 0.0
    assign = {}
    for base, ng in BLOCKS:
        for c in range(NCH):
            for k in range(ng):
                j = base + k
                full = j != NGR - 1
                dc = D_FULL if full else D_HALF
                ac = A_FULL if full else A_HALF
                if tD + dc <= tA + ac:
                    assign[(c, j)] = 'D'
                    tD += dc
                else:
                    assign[(c, j)] = 'A'
                    tA += ac
    dve_off, act_ord = {}, {}
    dve_w = act_n = 0
    for c in range(NCH):
        do = ao = 0
        for j in range(NGR):
            nsub = _gr_width(j) // SUB
            if assign[(c, j)] == 'D':
                dve_off[(c, j)] = do
                do += nsub
            else:
                act_ord[(c, j)] = ao
                ao += 1
        dve_w = max(dve_w, do)
        act_n = max(act_n, ao)
    return assign, dve_off, act_ord, dve_w, act_n


ASSIGN, DVE_OFF, ACT_ORD, DVE_W, ACT_N = _build_schedule()


def _build_bass():
    import concourse.bacc as bacc
    import concourse.tile as tile
    from concourse import mybir

    nc = bacc.Bacc("TRN2")
    f32 = mybir.dt.float32
    fp8 = mybir.dt.float8e4

    g_d = nc.dram_tensor("g", [128, 2, NPC_PAD], fp8, kind="ExternalInput")
    q_d = nc.dram_tensor("q", [128, 2, B], fp8, kind="ExternalInput")
    tau_d = nc.dram_tensor("tau", [128, NCH], f32, kind="ExternalInput")
    dve_d = nc.dram_tensor("dve", [128, NCH * DVE_W], f32,
                           kind="ExternalOutput")
    rel_d = nc.dram_tensor("rel", [128, NCH, ACT_N, GR], fp8,
                           kind="ExternalOutput")

    X = mybir.AxisListType.X
    MAX = mybir.AluOpType.max
    RELU = mybir.ActivationFunctionType.Relu
    pm = mybir.MatmulPerfMode.DoubleRow

    with tile.TileContext(nc) as tc:
        with tc.tile_pool(name="qp", bufs=1) as qp, \
             tc.tile_pool(name="gp", bufs=1) as gp, \
             tc.tile_pool(name="op", bufs=1) as op, \
             tc.tile_pool(name="rp", bufs=1) as rp, \
             tc.tile_pool(name="pp", bufs=1, space="PSUM") as pp:
            q8 = qp.tile([128, 2, B], fp8, tag="q8")
            tau = qp.tile([128, NCH], f32, tag="tau")
            gal = []
            for i, (g0, ng) in enumerate(BLOCKS):
                c0 = g0 * GR
                cw = sum(_gr_width(g0 + k) for k in range(ng))
                t = gp.tile([128, 2, cw], fp8, tag=f"gal{i}", name=f"gal_t{i}")
                gal.append((t, g0, c0, cw))
            t0, _, c00, cw0 = gal[0]
            nc.sync.dma_start(out=t0[:], in_=g_d[:, :, c00:c00 + cw0])
            nc.sync.dma_start(out=q8[:, :, 0:128], in_=q_d[:, :, 0:128])
            nc.sync.dma_start(out=q8[:, :, 128:B], in_=q_d[:, :, 128:B])
            nc.sync.dma_start(out=tau[:], in_=tau_d[:])

            dve_o = op.tile([128, NCH * DVE_W], f32, tag="dve_o")

            relmax = max(ng for _, ng in BLOCKS)
            for i, (gbase, ng) in enumerate(BLOCKS):
                t, _, cbase, _ = gal[i]
                if i + 1 < len(BLOCKS):
                    tn, _, cn0, cnw = gal[i + 1]
                    nc.sync.dma_start(out=tn[:],
                                      in_=g_d[:, :, cn0:cn0 + cnw])
                for c in range(NCH):
                    lhs = q8[:, :, c * 128:(c + 1) * 128]
                    acts = [gbase + k for k in range(ng)
                            if ASSIGN[(c, gbase + k)] == 'A']
                    rt = None
                    if acts:
                        rt = rp.tile([128, relmax * GR], fp8,
                                     tag="rel", bufs=6,
                                     name=f"rel_t{i}_{c}")
                    li = 0
                    for k in range(ng):
                        j = gbase + k
                        w = _gr_width(j)
                        l0 = j * GR - cbase
                        ps = pp.tile([128, GR], f32, tag="ps", bufs=4)
                        for h0 in range(0, w, 512):
                            nc.tensor.matmul(
                                ps[:, h0:h0 + 512], lhs,
                                t[:, :, l0 + h0:l0 + h0 + 512],
                                start=True, stop=True, perf_mode=pm)
                        if ASSIGN[(c, j)] == 'D':
                            nsub = w // SUB
                            d0 = c * DVE_W + DVE_OFF[(c, j)]
                            nc.vector.tensor_reduce(
                                dve_o[:, d0:d0 + nsub],
                                ps[:, :w].rearrange("p (r w) -> p r w",
                                                    r=nsub),
                                axis=X, op=MAX)
                        else:
                            if w < GR:
                                # zero the pad tail so host sees no noise
                                nc.scalar.activation(
                                    out=rt[:, li * GR + w:(li + 1) * GR],
                                    in_=ps[:, :GR - w], func=RELU,
                                    bias=tau[:, c:c + 1], scale=0.0)
                            nc.scalar.activation(
                                out=rt[:, li * GR:li * GR + w],
                                in_=ps[:, :w], func=RELU,
                                bias=tau[:, c:c + 1], scale=1.0)
                            li += 1
                    if acts:
                        o0 = ACT_ORD[(c, acts[0])]
                        nc.sync.dma_start(
                            out=rel_d[:, c, o0:o0 + len(acts)],
                            in_=rt[:, :len(acts) * GR].rearrange(
                                "p (a g) -> p a g", a=len(acts)))
                    if i == len(BLOCKS) - 1 and c == 3:
                        h = (NCH // 2) * DVE_W
                        nc.sync.dma_start(out=dve_d[:, :h],
                                          in_=dve_o[:, :h])
            h = (NCH // 2) * DVE_W
            nc.sync.dma_start(out=dve_d[:, h:], in_=dve_o[:, h:])
    if not nc.is_finalized():
        nc.finalize()
    return nc


def _run_device(g_shards, q_packed, tau_packed):
    from concourse.bass_utils import run_bass_kernel_spmd
    if "nc" not in _CACHE:
        _CACHE["nc"] = _build_bass()
    nc = _CACHE["nc"]
    in_maps = [{"g": g_shards[c], "q": q_packed, "tau": tau_packed}
               for c in range(NCORES)]
    res = run_bass_kernel_spmd(nc, in_maps, list(range(NCORES)))
    return ([res.results[c]["dve"] for c in range(NCORES)],
            [res.results[c]["rel"] for c in range(NCORES)])


def _run_emulated(g_shards, q_packed, tau_packed):
    import ml_dtypes
    FP8 = ml_dtypes.float8_e4m3fn
    qf = q_packed.astype(np.float32)
    dves, rels = [], []
    for core in range(NCORES):
        gf = g_shards[core].astype(np.float32)
        sim = np.einsum("pib,pin->bn", qf, gf)   # [B, NPC_PAD]
        dve = np.zeros((128, NCH * DVE_W), np.float32)
        rel = np.zeros((128, NCH, ACT_N, GR), dtype=FP8)
        for c in range(NCH):
            sc = sim[c * 128:(c + 1) * 128]
            for j in range(NGR):
                w = _gr_width(j)
                gsl = sc[:, j * GR:j * GR + w]
                if ASSIGN[(c, j)] == 'D':
                    nsub = w // SUB
                    d0 = c * DVE_W + DVE_OFF[(c, j)]
                    dve[:, d0:d0 + nsub] = \
                        gsl.reshape(128, nsub, SUB).max(axis=2)
                else:
                    bias = tau_packed[:, c:c + 1]
                    rel[:, c, ACT_ORD[(c, j)], :w] = \
                        np.maximum(gsl + bias, 0).astype(FP8)
        dves.append(dve)
        rels.append(rel)
    return dves, rels


def kernel(test_features, train_features, train_labels):
    import ml_dtypes
    FP8 = ml_dtypes.float8_e4m3fn

    test_features = np.asarray(test_features, dtype=np.float32)
    train_features = np.asarray(train_features, dtype=np.float32)
    labels = np.asarray(train_labels).astype(np.int64)

    tf64 = train_features.astype(np.float64)
    norm_d = np.maximum(np.sqrt(np.sum(tf64 * tf64, axis=0)), EPS)
    q64 = test_features.astype(np.float64)
    qn = np.sqrt(np.sum(q64 * q64, axis=1, keepdims=True))
    q_scaled = q64 / np.maximum(qn, EPS) / norm_d

    sigma_b = np.sqrt(np.sum(q_scaled * q_scaled, axis=1))
    s_b = 128.0 / sigma_b
    q8 = (q_scaled * s_b[:, None]).astype(FP8)
    g8 = train_features.T.astype(FP8)

    q8f = q8.astype(np.float64)
    g8_sq_mean = float(np.mean(g8.astype(np.float32) ** 2))
    sig_dev = np.sqrt(np.sum(q8f * q8f, axis=1) * g8_sq_mean)
    tau_dev = TAU_Z * sig_dev

    q_packed = np.ascontiguousarray(
        q8.T.reshape(2, 128, B).transpose(1, 0, 2))
    g_shards = []
    for core in range(NCORES):
        sl = np.zeros((2, 128, NPC_PAD), dtype=FP8)
        sl[:, :, :NPC] = g8[:, core * NPC:(core + 1) * NPC].reshape(2, 128, NPC)
        g_shards.append(np.ascontiguousarray(sl.transpose(1, 0, 2)))
    tau_packed = np.ascontiguousarray(
        (-tau_dev).astype(np.float32).reshape(NCH, 128).T)

    if os.environ.get("KNN_EMULATE"):
        dves, rels = _run_emulated(g_shards, q_packed, tau_packed)
    else:
        dves, rels = _run_device(g_shards, q_packed, tau_packed)

    # ---- host screen ----
    NBLK = NPC_PAD // SUB
    flags = np.zeros((B, NCORES, NBLK), dtype=bool)
    percol = [[] for _ in range(B)]
    act_j_of = np.full((NCH, ACT_N), -1, np.int64)
    for (c, j), o in ACT_ORD.items():
        act_j_of[c, o] = j
    for core in range(NCORES):
        dve = dves[core].astype(np.float64)      # [128, NCH*DVE_W]
        relbytes = np.ascontiguousarray(rels[core]).view(np.uint8)
        for c in range(NCH):
            brow = c * 128
            tt = tau_dev[brow:brow + 128]
            for j in range(NGR):
                if ASSIGN[(c, j)] != 'D':
                    continue
                w = _gr_width(j)
                nsub = w // SUB
                d0 = c * DVE_W + DVE_OFF[(c, j)]
                k0 = (j * GR) // SUB
                m = dve[:, d0:d0 + nsub]
                flags[brow:brow + 128, core, k0:k0 + nsub] |= \
                    m >= tt[:, None]
            nz_p, nz_o, nz_x = np.nonzero(relbytes[:, c])
            if len(nz_p):
                jj = act_j_of[c][nz_o]
                loc = jj * GR + nz_x
                valid = (jj >= 0) & (loc < NPC)
                gcols = core * NPC + loc
                for p, col in zip(nz_p[valid], gcols[valid]):
                    percol[brow + p].append(col)

    flags = flags.reshape(B, NCORES * NBLK)
    seg_queries = [np.nonzero(flags[:, s])[0] for s in range(NCORES * NBLK)]

    per_q_vals = [[] for _ in range(B)]
    per_q_cols = [[] for _ in range(B)]
    for s, qs in enumerate(seg_queries):
        if len(qs) == 0:
            continue
        core, k = divmod(s, NBLK)
        c0 = core * NPC + SUB * k
        c1 = core * NPC + min(SUB * k + SUB, NPC)
        if c0 >= c1:
            continue
        block = tf64[c0:c1]
        sims = q_scaled[qs] @ block.T
        cols = np.arange(c0, c1)
        for i, b in enumerate(qs):
            per_q_vals[b].append(sims[i])
            per_q_cols[b].append(cols)

    scores = np.zeros((B, NUM_CLASSES), dtype=np.float64)
    fallback = []
    for b in range(B):
        vs = per_q_vals[b]
        cs = per_q_cols[b]
        if percol[b]:
            pc = np.asarray(percol[b], dtype=np.int64)
            vs = vs + [tf64[pc] @ q_scaled[b]]
            cs = cs + [pc]
        if vs:
            v = np.concatenate(vs)
            cidx = np.concatenate(cs)
        else:
            v = np.empty(0)
            cidx = np.empty(0, np.int64)
        if len(v) < NB_KNN:
            fallback.append(b)
            continue
        sel = np.argpartition(-v, NB_KNN - 1)[:NB_KNN]
        v10_dev = s_b[b] * np.sort(v[sel])[0]
        if v10_dev <= tau_dev[b] + CERT_Z * sig_dev[b]:
            fallback.append(b)
            continue
        order = np.lexsort((cidx[sel], -v[sel]))
        sel = sel[order]
        topv = v[sel]
        w = np.exp(topv / T - np.max(topv) / T)
        w /= w.sum()
        np.add.at(scores[b], labels[cidx[sel]], w)

    if fallback:
        fb = np.asarray(fallback)
        sims = q_scaled[fb] @ tf64.T
        for i, b in enumerate(fb):
            v = sims[i]
            sel = np.argpartition(-v, NB_KNN - 1)[:NB_KNN]
            order = np.lexsort((sel, -v[sel]))
            sel = sel[order]
            topv = v[sel]
            w = np.exp(topv / T - np.max(topv) / T)
            w /= w.sum()
            np.add.at(scores[b], labels[sel], w)

    return scores.astype(np.float32)


if __name__ == "__main__":
    rng = np.random.default_rng(0)
    tf = rng.standard_normal((B, D), dtype=np.float32)
    trf = rng.standard_normal((N, D), dtype=np.float32)
    trl = rng.integers(0, NUM_CLASSES, N).astype(np.int64)
    os.environ["KNN_EMULATE"] = "1"
    out = kernel(tf, trf, trl)
    print(out.shape, out.dtype, out.sum())
